# revision 13
# baseline (speedup 1.0000x reference)
"""Trainium2 Bass kernel for nn_ContrastiveLoss (B=4096, D=256, margin=1.0).

Math (exact restructuring of the reference):
  loss = [ sum_{i<j, same} 0.5*(d2_ij + 1e-8)
         + sum_{i<j, diff} 0.5*relu(1 - d_ij)^2 ] / (B(B-1)/2 + 1e-8)

  Similar-pair term has a closed form per class c:
     sum_{i<j in c} d2 = n_c * sum_sq_c - ||sum_e_c||^2
  so it only needs class sums / sum of squared norms (computed on device).

  Dissimilar term needs elementwise distances only on the mixed-label
  rectangle (n_small x n_large).  Rows sorted by label on host; each of the
  8 cores gets a (rows/2 x cols/4) block.  An augmented GEMM (K = 256 + 2)
  puts -0.5*d2 directly in PSUM; DVE computes t = min(d2, 1) with a fused
  row-sum; ACT computes sqrt(t + 1e-8) with a fused row-sum.  Using
  (s-1)^2 = (t+1e-8) - 2s + 1 the full sum of relu(1-d)^2 is recovered from
  the two accumulators, with a device-side calibration of s1_hat =
  ACT_sqrt(1 + 1e-8) so that clamped elements (d >= 1) contribute exactly 0.

Pad rows use zero embeddings (class sums unpolluted) and get +BIG added to
their squared norm via the augmentation row, pushing their distances far
above the margin so they contribute exactly 0 to the rectangle term.
"""

import sys
import os

for _p in ("/opt/trn_rl_repo", "/root/.axon_site/_ro/trn_rl_repo"):
    if os.path.isdir(_p) and _p not in sys.path:
        sys.path.insert(0, _p)

import numpy as np

B_FULL, D = 4096, 256
MARGIN = 1.0
EPS = 1e-8
BIG = 1.0e4
R_CAP, C_CAP = 2048, 3072      # padded small-class rows / large-class cols
RSH, CSH = 2, 4                # core grid: row-shards x col-shards
AR = R_CAP // RSH              # 1024 rows per core
BC = C_CAP // CSH              # 768 cols per core
KAUG = D + 2                   # 258
N_CORES = 8
MT = AR // 128                 # 8 M-tiles per core

_PROGRAM = None


def _build_program():
    import concourse.bacc as bacc
    import concourse.tile as tile
    from concourse import mybir

    f32 = mybir.dt.float32
    f32r = mybir.dt.float32r
    mult = mybir.AluOpType.mult
    amin = mybir.AluOpType.min
    aadd = mybir.AluOpType.add
    Sqrt = mybir.ActivationFunctionType.Sqrt
    Copy = mybir.ActivationFunctionType.Copy

    nc = bacc.Bacc("TRN2", target_bir_lowering=False, debug=False,
                   num_devices=N_CORES)
    bf16 = mybir.dt.bfloat16
    a_dram = nc.dram_tensor("a_t", [KAUG, AR], bf16, kind="ExternalInput").ap()
    b_dram = nc.dram_tensor("b_t", [KAUG, BC], bf16, kind="ExternalInput").ap()
    asq_dram = nc.dram_tensor("asq", [1, AR], f32, kind="ExternalInput").ap()
    bsq_dram = nc.dram_tensor("bsq", [1, BC], f32, kind="ExternalInput").ap()
    out_dram = nc.dram_tensor("out", [128, 32], f32, kind="ExternalOutput").ap()

    with tile.TileContext(nc) as tc:
        with (
            tc.tile_pool(name="big", bufs=1) as big,
            tc.tile_pool(name="work", bufs=3) as work,
            tc.tile_pool(name="junk", bufs=2) as junkp,
            tc.tile_pool(name="psum", bufs=3, space="PSUM") as psum,
            tc.tile_pool(name="psum1", bufs=1, space="PSUM") as psum1,
        ):
            a0 = big.tile([128, AR], bf16, tag="a0")
            a1 = big.tile([128, AR], bf16, tag="a1")
            aaug = big.tile([2, AR], bf16, tag="aaug")
            b0 = big.tile([128, BC], bf16, tag="b0")
            b1 = big.tile([128, BC], bf16, tag="b1")
            baug = big.tile([2, BC], bf16, tag="baug")
            outs = big.tile([128, 32], f32, tag="outs")
            epsb = big.tile([128, 1], f32, tag="epsb")
            cal = big.tile([1, 8], f32, tag="cal")
            ones = big.tile([128, 1], bf16, tag="ones")

            nc.gpsimd.memset(outs[:], 0.0)
            nc.gpsimd.memset(epsb[:], EPS)
            nc.gpsimd.memset(cal[:], 1.0)
            nc.gpsimd.memset(ones[:], 1.0)

            nc.sync.dma_start(a0[:], a_dram[0:128, :])
            nc.sync.dma_start(a1[:], a_dram[128:256, :])
            nc.sync.dma_start(aaug[:], a_dram[256:258, :])
            nc.sync.dma_start(b0[:], b_dram[0:128, :])
            nc.sync.dma_start(b1[:], b_dram[128:256, :])
            nc.sync.dma_start(baug[:], b_dram[256:258, :])
            asqrow = big.tile([1, AR], f32, tag="asqrow")
            nc.sync.dma_start(asqrow[:], asq_dram[:])
            bsqrow = big.tile([1, BC], f32, tag="bsqrow")
            nc.sync.dma_start(bsqrow[:], bsq_dram[:])

            # ---- main rectangle: PSUM holds -0.5 * d2_augmented ----
            pst = psum1.tile([1, BC], f32, tag="pst")  # column sums of t
            for m in range(MT):
                ms = slice(m * 128, (m + 1) * 128)
                ps = psum.tile([128, BC], f32, tag="ps")
                for cs in (slice(0, 512), slice(512, BC)):
                    nc.tensor.matmul(ps[:, cs], a0[:, ms],
                                     b0[:, cs],
                                     start=True, stop=False)
                    nc.tensor.matmul(ps[:, cs], a1[:, ms],
                                     b1[:, cs],
                                     start=False, stop=False)
                    nc.tensor.matmul(ps[:, cs], aaug[:, ms],
                                     baug[:, cs],
                                     start=False, stop=True)
                # t = min(-2 * psum, 1) = min(d2, 1)   (DVE, elementwise)
                t = work.tile([128, BC], bf16, tag="t")
                nc.vector.tensor_scalar(t[:], ps[:], -2.0, 1.0, mult, amin)
                # s = sqrt(t + eps);  accum_out = per-partition row sums
                sj = work.tile([128, BC], bf16, tag="sj")
                nc.scalar.activation(sj[:], t[:], Sqrt, bias=epsb[:],
                                     scale=1.0, accum_out=outs[:, m:m + 1])
                # column sums of t accumulate on PE: pst += ones.T @ t
                for cs in (slice(0, 512), slice(512, BC)):
                    nc.tensor.matmul(pst[:, cs], ones[:],
                                     t[:, cs],
                                     start=(m == 0), stop=(m == MT - 1))
            # total sum of t -> outs[0, 16]
            jt = junkp.tile([1, BC], f32, tag="jt")
            nc.vector.tensor_scalar(jt[:], pst[:], 1.0, None, mult, aadd,
                                    accum_out=outs[0:1, 16:17])

            # ---- moments ----
            # class sums (a side on ACT, b side on DVE, to balance engines)
            ja = junkp.tile([128, AR], bf16, tag="ja")
            nc.scalar.activation(ja[:], a0[:], Copy,
                                 accum_out=outs[:, 24:25])
            ja2 = junkp.tile([128, AR], bf16, tag="ja")
            nc.scalar.activation(ja2[:], a1[:], Copy,
                                 accum_out=outs[:, 25:26])
            jb = junkp.tile([128, BC], bf16, tag="jb")
            nc.vector.tensor_scalar(jb[:], b0[:], 1.0, None, mult, aadd,
                                    accum_out=outs[:, 26:27])
            jb2 = junkp.tile([128, BC], bf16, tag="jb")
            nc.vector.tensor_scalar(jb2[:], b1[:], 1.0, None, mult, aadd,
                                    accum_out=outs[:, 27:28])
            # sums of the scaled-sq rows (partition 0 for a, partition 1 for b)
            jra = junkp.tile([1, AR], f32, tag="jra")
            nc.vector.tensor_scalar(jra[:], asqrow[:], 1.0, None, mult,
                                    aadd, accum_out=outs[0:1, 28:29])
            jrb = junkp.tile([1, BC], f32, tag="jrb")
            nc.vector.tensor_scalar(jrb[:], bsqrow[:], 1.0, None, mult,
                                    aadd, accum_out=outs[0:1, 17:18])
            # calibration: s1_hat = ACT_sqrt(1 + eps), accumulated over 8 ones
            jcal = junkp.tile([1, 8], f32, tag="jcal")
            nc.scalar.activation(jcal[:], cal[:], Sqrt, bias=epsb[0:1, :],
                                 scale=1.0, accum_out=outs[0:1, 29:30])

            nc.sync.dma_start(out_dram[:], outs[:])
    nc.compile()
    return nc


def get_program():
    global _PROGRAM
    if _PROGRAM is None:
        _PROGRAM = _build_program()
    return _PROGRAM


def build_in_maps(emb, lab):
    """Host-side sharding prep. Returns (in_maps, meta) or None if the label
    split exceeds the compiled caps."""
    idx0 = np.nonzero(lab == 0)[0]
    idx1 = np.nonzero(lab == 1)[0]
    if len(idx0) <= len(idx1):
        idxs, idxl = idx0, idx1
    else:
        idxs, idxl = idx1, idx0
    ns, nl = len(idxs), len(idxl)
    if ns > R_CAP or nl > C_CAP:
        return None
    Es = emb[idxs]                      # (ns, 256)
    El = emb[idxl]                      # (nl, 256)
    sqs = np.einsum('ij,ij->i', Es.astype(np.float64), Es.astype(np.float64))
    sql = np.einsum('ij,ij->i', El.astype(np.float64), El.astype(np.float64))

    import ml_dtypes
    bf16 = ml_dtypes.bfloat16

    A = np.zeros((KAUG, R_CAP), np.float32)
    A[:D, :ns] = Es.T
    A[D, :ns] = (-0.5 * sqs).astype(np.float32)
    A[D, ns:] = -0.5 * BIG
    A[D + 1, :] = 1.0

    Bt = np.zeros((KAUG, C_CAP), np.float32)
    Bt[:D, :nl] = El.T
    Bt[D, :] = 1.0
    Bt[D + 1, :nl] = (-0.5 * sql).astype(np.float32)
    Bt[D + 1, nl:] = -0.5 * BIG

    A_bf = A.astype(bf16)
    Bt_bf = Bt.astype(bf16)

    in_maps = []
    for ri in range(RSH):
        for ci in range(CSH):
            in_maps.append({
                "a_t": np.ascontiguousarray(A_bf[:, ri * AR:(ri + 1) * AR]),
                "b_t": np.ascontiguousarray(Bt_bf[:, ci * BC:(ci + 1) * BC]),
                "asq": np.ascontiguousarray(A[D:D + 1, ri * AR:(ri + 1) * AR]),
                "bsq": np.ascontiguousarray(
                    Bt[D + 1:D + 2, ci * BC:(ci + 1) * BC]),
            })
    return in_maps, (ns, nl)


def combine(outs_list, ns, nl):
    """Combine per-core (128, 32) outputs into the scalar loss (float64)."""
    o = [np.asarray(x, np.float64) for x in outs_list]
    n_elem = float(R_CAP) * float(C_CAP)

    Ts = sum(ok[:, 0:8].sum() for ok in o)
    Tt = sum(ok[0, 16] for ok in o)
    s1_hat = o[0][0, 29] / 8.0
    # sum over rectangle of relu(1 - d)^2, exactly 0 for clamped elements
    t2_total = (Tt - n_elem) + 2.0 * (n_elem * s1_hat - Ts)

    S_small = np.zeros(D)
    for ri in range(RSH):
        ok = o[ri * CSH + 0]
        S_small[0:128] += ok[:, 24]
        S_small[128:256] += ok[:, 25]
    S_large = np.zeros(D)
    for ci in range(CSH):
        ok = o[ci]
        S_large[0:128] += ok[:, 26]
        S_large[128:256] += ok[:, 27]

    sum_sq_small = sum(o[ri * CSH][0, 28] for ri in range(RSH)) * (-2.0) \
        - BIG * (R_CAP - ns)
    sum_sq_large = sum(o[ci][0, 17] for ci in range(CSH)) * (-2.0) \
        - BIG * (C_CAP - nl)

    term1_d2 = (ns * sum_sq_small - S_small @ S_small
                + nl * sum_sq_large - S_large @ S_large)
    n_same = ns * (ns - 1) / 2.0 + nl * (nl - 1) / 2.0
    num = 0.5 * (term1_d2 + EPS * n_same) + 0.5 * t2_total
    den = B_FULL * (B_FULL - 1) / 2.0 + EPS
    return num / den


def _numpy_fallback(emb, lab):
    e = emb.astype(np.float64)
    sq = (e * e).sum(1)
    gram = e @ e.T
    d2 = np.maximum(sq[:, None] + sq[None, :] - 2.0 * gram, 0.0)
    dist = np.sqrt(d2 + EPS)
    same = (lab[:, None] == lab[None, :]).astype(np.float64)
    loss = same * 0.5 * dist ** 2 \
        + (1.0 - same) * 0.5 * np.maximum(MARGIN - dist, 0.0) ** 2
    mask = np.triu(np.ones_like(loss), k=1)
    return (loss * mask).sum() / (mask.sum() + EPS)


def run_device(in_maps, trace=False, **kw):
    from concourse.bass_utils import run_bass_kernel_spmd
    nc = get_program()
    return run_bass_kernel_spmd(nc, in_maps, list(range(N_CORES)),
                                trace=trace, **kw)


def kernel(embeddings, labels):
    emb = np.ascontiguousarray(np.asarray(embeddings), dtype=np.float32)
    lab = np.asarray(labels).astype(np.int64).ravel()
    ok_shapes = (emb.shape == (B_FULL, D) and lab.shape == (B_FULL,)
                 and np.all((lab == 0) | (lab == 1)))
    prep = build_in_maps(emb, lab) if ok_shapes else None
    if prep is None:
        return np.float32(_numpy_fallback(emb, lab))
    in_maps, (ns, nl) = prep
    res = run_device(in_maps)
    outs_list = [res.results[k]["out"] for k in range(N_CORES)]
    loss = combine(outs_list, ns, nl)
    return np.float32(loss)


# revision 15
# speedup vs baseline: 1.2293x; 1.2293x over previous
"""Trainium2 Bass kernel for nn_ContrastiveLoss (B=4096, D=256, margin=1.0).

Math (exact restructuring of the reference):
  loss = [ sum_{i<j, same} 0.5*(d2_ij + 1e-8)
         + sum_{i<j, diff} 0.5*relu(1 - d_ij)^2 ] / (B(B-1)/2 + 1e-8)

  The similar-pair term has a closed form per class c:
     sum_{i<j in c} d2 = n_c * sum_sq_c - ||sum_e_c||^2
  so it only needs class sums / summed squared norms (computed on device).

  The dissimilar term needs elementwise distances only on the mixed-label
  (n_small x n_large) rectangle.  Rows are sorted by label on host; each of
  the 8 cores gets a (R_CAP/2 x C_CAP/4) block.  relu(1-d)^2 is EXACTLY zero
  unless some mixed pair has d2 < 1, so the fast program only has to PROVE
  no pair violates the margin: the GEMM leaves psum = dot_ij - 0.5*sq_i and
  a single DVE tensor_scalar per psum block computes
     accum[j] = max( max_i(psum[j,i] - 0.5*sq_j), -1.0 )   (= -0.5*min(d2,2))
  If every accum <= -0.7 (d2_min >= 1.4 with margin for bf16 noise), the
  dissimilar term is exactly 0.  Otherwise a full fallback program (sqrt
  pipeline, compiled lazily) recomputes it exactly.

Pad rows use zero embeddings (class sums unpolluted) and get +BIG added to
their squared norm via the augmentation terms, pushing their distances far
above the margin.
"""

import sys
import os

for _p in ("/opt/trn_rl_repo", "/root/.axon_site/_ro/trn_rl_repo"):
    if os.path.isdir(_p) and _p not in sys.path:
        sys.path.insert(0, _p)

import numpy as np

B_FULL, D = 4096, 256
MARGIN = 1.0
EPS = 1e-8
BIG = 1.0e4
R_CAP, C_CAP = 2048, 3072      # padded small-class rows / large-class cols
RSH, CSH = 2, 4                # core grid: row-shards x col-shards
AR = R_CAP // RSH              # 1024 rectangle rows per core (free axis)
BC = C_CAP // CSH              # 768 rectangle cols per core (partition axis)
NBLK = BC // 128               # 6 psum blocks per core
N_CORES = 8

# detection threshold: trigger the exact fallback if min mixed d2 < 1.4
DETECT_ACCUM_THRESH = -0.7

_PROGRAMS = {}


def _build_detect_program():
    """Fast path: GEMM + margin-violation detection + moments."""
    import concourse.bacc as bacc
    import concourse.tile as tile
    from concourse import mybir

    f32 = mybir.dt.float32
    bf16 = mybir.dt.bfloat16
    mult = mybir.AluOpType.mult
    amax = mybir.AluOpType.max
    asub = mybir.AluOpType.subtract
    aadd = mybir.AluOpType.add
    Copy = mybir.ActivationFunctionType.Copy

    nc = bacc.Bacc("TRN2", target_bir_lowering=False, debug=False,
                   num_devices=N_CORES)
    a_dram = nc.dram_tensor("a_t", [D + 1, AR], bf16, kind="ExternalInput").ap()
    b_dram = nc.dram_tensor("b_t", [D, BC], bf16, kind="ExternalInput").ap()
    bsqc_dram = nc.dram_tensor("bsqc", [128, NBLK], f32,
                               kind="ExternalInput").ap()
    asq_dram = nc.dram_tensor("asq", [1, AR], f32, kind="ExternalInput").ap()
    bsq_dram = nc.dram_tensor("bsq", [1, BC], f32, kind="ExternalInput").ap()
    out_dram = nc.dram_tensor("out", [128, 32], f32, kind="ExternalOutput").ap()

    with tile.TileContext(nc) as tc:
        with (
            tc.tile_pool(name="big", bufs=1) as big,
            tc.tile_pool(name="junk", bufs=2) as junkp,
            tc.tile_pool(name="psum", bufs=3, space="PSUM") as psum,
        ):
            a0 = big.tile([128, AR], bf16, tag="a0")
            a1 = big.tile([128, AR], bf16, tag="a1")
            zrow = big.tile([1, AR], bf16, tag="zrow")
            b0 = big.tile([128, BC], bf16, tag="b0")
            b1 = big.tile([128, BC], bf16, tag="b1")
            bsqc = big.tile([128, NBLK], f32, tag="bsqc")
            asqrow = big.tile([1, AR], f32, tag="asqrow")
            bsqrow = big.tile([1, BC], f32, tag="bsqrow")
            onesr = big.tile([1, 128], bf16, tag="onesr")
            outs = big.tile([128, 32], f32, tag="outs")

            nc.gpsimd.memset(outs[:], 0.0)
            nc.gpsimd.memset(onesr[:], 1.0)

            nc.sync.dma_start(a0[:], a_dram[0:128, :])
            nc.sync.dma_start(a1[:], a_dram[128:256, :])
            nc.sync.dma_start(zrow[:], a_dram[256:257, :])
            nc.sync.dma_start(b0[:], b_dram[0:128, :])
            nc.sync.dma_start(b1[:], b_dram[128:256, :])
            nc.sync.dma_start(bsqc[:], bsqc_dram[:])
            nc.sync.dma_start(asqrow[:], asq_dram[:])
            nc.sync.dma_start(bsqrow[:], bsq_dram[:])

            # psum[j, i] = dot_ij - 0.5*sq_i - 0.5*Abias_i   (j: block cols)
            for blk in range(NBLK):
                bs = slice(blk * 128, (blk + 1) * 128)
                ps = psum.tile([128, AR], f32, tag="ps")
                for hs in (slice(0, 512), slice(512, AR)):
                    nc.tensor.matmul(ps[:, hs], b0[:, bs], a0[:, hs],
                                     start=True, stop=False)
                    nc.tensor.matmul(ps[:, hs], b1[:, bs], a1[:, hs],
                                     start=False, stop=False)
                    nc.tensor.matmul(ps[:, hs], onesr[:], zrow[:, hs],
                                     start=False, stop=True)
                # accum[j] = max( max_i(psum - 0.5*(sq_j+Bbias_j)), -1.0 )
                jd = junkp.tile([128, AR], f32, tag="jd")
                nc.vector.tensor_scalar(jd[:], ps[:], bsqc[:, blk:blk + 1],
                                        -1.0, asub, amax,
                                        accum_out=outs[:, blk:blk + 1])

            # ---- moments (ACT engine; DVE is busy with the detection) ----
            ja = junkp.tile([128, AR], bf16, tag="ja")
            nc.scalar.activation(ja[:], a0[:], Copy, accum_out=outs[:, 24:25])
            ja2 = junkp.tile([128, AR], bf16, tag="ja")
            nc.scalar.activation(ja2[:], a1[:], Copy, accum_out=outs[:, 25:26])
            jb = junkp.tile([128, BC], bf16, tag="jb")
            nc.scalar.activation(jb[:], b0[:], Copy, accum_out=outs[:, 26:27])
            jb2 = junkp.tile([128, BC], bf16, tag="jb")
            nc.scalar.activation(jb2[:], b1[:], Copy, accum_out=outs[:, 27:28])
            jra = junkp.tile([1, AR], f32, tag="jra")
            nc.scalar.activation(jra[:], asqrow[:], Copy,
                                 accum_out=outs[0:1, 28:29])
            jrb = junkp.tile([1, BC], f32, tag="jrb")
            nc.scalar.activation(jrb[:], bsqrow[:], Copy,
                                 accum_out=outs[0:1, 17:18])

            nc.sync.dma_start(out_dram[:], outs[:])
    nc.compile()
    return nc


def _build_full_program():
    """Exact fallback: full min/sqrt pipeline for the dissimilar term.
    Only compiled + run when the detect program finds d2_min < 1.4."""
    import concourse.bacc as bacc
    import concourse.tile as tile
    from concourse import mybir

    f32 = mybir.dt.float32
    bf16 = mybir.dt.bfloat16
    mult = mybir.AluOpType.mult
    amin = mybir.AluOpType.min
    aadd = mybir.AluOpType.add
    Sqrt = mybir.ActivationFunctionType.Sqrt

    nc = bacc.Bacc("TRN2", target_bir_lowering=False, debug=False,
                   num_devices=N_CORES)
    a_dram = nc.dram_tensor("a_t", [D + 1, AR], bf16, kind="ExternalInput").ap()
    b_dram = nc.dram_tensor("b_t", [D, BC], bf16, kind="ExternalInput").ap()
    bsqc_dram = nc.dram_tensor("bsqc", [128, NBLK], f32,
                               kind="ExternalInput").ap()
    out_dram = nc.dram_tensor("out", [128, 32], f32, kind="ExternalOutput").ap()

    with tile.TileContext(nc) as tc:
        with (
            tc.tile_pool(name="big", bufs=1) as big,
            tc.tile_pool(name="work", bufs=3) as work,
            tc.tile_pool(name="junk", bufs=2) as junkp,
            tc.tile_pool(name="psum", bufs=3, space="PSUM") as psum,
        ):
            a0 = big.tile([128, AR], bf16, tag="a0")
            a1 = big.tile([128, AR], bf16, tag="a1")
            zrow = big.tile([1, AR], bf16, tag="zrow")
            b0 = big.tile([128, BC], bf16, tag="b0")
            b1 = big.tile([128, BC], bf16, tag="b1")
            bsqc = big.tile([128, NBLK], f32, tag="bsqc")
            onesr = big.tile([1, 128], bf16, tag="onesr")
            epsb = big.tile([128, 1], f32, tag="epsb")
            cal = big.tile([1, 8], f32, tag="cal")
            outs = big.tile([128, 32], f32, tag="outs")

            nc.gpsimd.memset(outs[:], 0.0)
            nc.gpsimd.memset(onesr[:], 1.0)
            nc.gpsimd.memset(epsb[:], EPS)
            nc.gpsimd.memset(cal[:], 1.0)

            nc.sync.dma_start(a0[:], a_dram[0:128, :])
            nc.sync.dma_start(a1[:], a_dram[128:256, :])
            nc.sync.dma_start(zrow[:], a_dram[256:257, :])
            nc.sync.dma_start(b0[:], b_dram[0:128, :])
            nc.sync.dma_start(b1[:], b_dram[128:256, :])
            nc.sync.dma_start(bsqc[:], bsqc_dram[:])

            for blk in range(NBLK):
                bs = slice(blk * 128, (blk + 1) * 128)
                ps = psum.tile([128, AR], f32, tag="ps")
                for hs in (slice(0, 512), slice(512, AR)):
                    nc.tensor.matmul(ps[:, hs], b0[:, bs], a0[:, hs],
                                     start=True, stop=False)
                    nc.tensor.matmul(ps[:, hs], b1[:, bs], a1[:, hs],
                                     start=False, stop=False)
                    nc.tensor.matmul(ps[:, hs], onesr[:], zrow[:, hs],
                                     start=False, stop=True)
                # t = min(d2, 1) = min(-2*(psum - 0.5*sqb_j), 1)
                #   = -2 * max(psum - 0.5*sqb_j, -0.5)
                u = work.tile([128, AR], f32, tag="u")
                nc.vector.tensor_scalar(u[:], ps[:], bsqc[:, blk:blk + 1],
                                        -0.5, mybir.AluOpType.subtract,
                                        mybir.AluOpType.max)
                t = work.tile([128, AR], bf16, tag="t")
                nc.vector.tensor_scalar(t[:], u[:], -2.0, None, mult, aadd,
                                        accum_out=outs[:, 8 + blk:9 + blk])
                # s = sqrt(t + eps); accum = row sums
                sj = work.tile([128, AR], bf16, tag="sj")
                nc.scalar.activation(sj[:], t[:], Sqrt, bias=epsb[:],
                                     scale=1.0,
                                     accum_out=outs[:, blk:blk + 1])
            # calibration: s1_hat = ACT_sqrt(1 + eps) summed over 8 ones
            jcal = junkp.tile([1, 8], f32, tag="jcal")
            nc.scalar.activation(jcal[:], cal[:], Sqrt, bias=epsb[0:1, :],
                                 scale=1.0, accum_out=outs[0:1, 29:30])

            nc.sync.dma_start(out_dram[:], outs[:])
    nc.compile()
    return nc


def _get_program(kind):
    if kind not in _PROGRAMS:
        _PROGRAMS[kind] = (_build_detect_program() if kind == "detect"
                           else _build_full_program())
    return _PROGRAMS[kind]


def build_in_maps(emb, lab):
    """Host-side sharding prep. Returns (in_maps, meta) or None if the
    label split exceeds the compiled caps."""
    import ml_dtypes
    bf16 = ml_dtypes.bfloat16

    idx0 = np.nonzero(lab == 0)[0]
    idx1 = np.nonzero(lab == 1)[0]
    if len(idx0) <= len(idx1):
        idxs, idxl = idx0, idx1
    else:
        idxs, idxl = idx1, idx0
    ns, nl = len(idxs), len(idxl)
    if ns > R_CAP or nl > C_CAP:
        return None
    Es = emb[idxs]                      # (ns, 256)  -> rectangle rows (free)
    El = emb[idxl]                      # (nl, 256)  -> rectangle cols (parts)
    sqs = np.einsum('ij,ij->i', Es.astype(np.float64), Es.astype(np.float64))
    sql = np.einsum('ij,ij->i', El.astype(np.float64), El.astype(np.float64))

    # a side: embeddings + z row  (z = -0.5*(sq + pad_bias))
    A = np.zeros((D + 1, R_CAP), np.float32)
    A[:D, :ns] = Es.T
    A[D, :ns] = (-0.5 * sqs).astype(np.float32)
    A[D, ns:] = -0.5 * BIG

    # b side: embeddings only; its sq goes in per-partition columns
    Bt = np.zeros((D, C_CAP), np.float32)
    Bt[:, :nl] = El.T
    bsq_flat = np.full((C_CAP,), 0.5 * BIG, np.float32)
    bsq_flat[:nl] = (0.5 * sql).astype(np.float32)

    A_bf = A.astype(bf16)
    Bt_bf = Bt.astype(bf16)

    in_maps = []
    for ri in range(RSH):
        for ci in range(CSH):
            bslice = bsq_flat[ci * BC:(ci + 1) * BC]
            in_maps.append({
                "a_t": np.ascontiguousarray(A_bf[:, ri * AR:(ri + 1) * AR]),
                "b_t": np.ascontiguousarray(Bt_bf[:, ci * BC:(ci + 1) * BC]),
                "bsqc": np.ascontiguousarray(
                    bslice.reshape(NBLK, 128).T.astype(np.float32)),
                "asq": np.ascontiguousarray(
                    A[D:D + 1, ri * AR:(ri + 1) * AR]),
                "bsq": np.ascontiguousarray(
                    (-bslice).reshape(1, BC)),
            })
    return in_maps, (ns, nl)


def combine_term1(outs_list, ns, nl):
    """Similar-pair closed form from the moment outputs (float64)."""
    o = [np.asarray(x, np.float64) for x in outs_list]
    S_small = np.zeros(D)
    for ri in range(RSH):
        ok = o[ri * CSH + 0]
        S_small[0:128] += ok[:, 24]
        S_small[128:256] += ok[:, 25]
    S_large = np.zeros(D)
    for ci in range(CSH):
        ok = o[ci]
        S_large[0:128] += ok[:, 26]
        S_large[128:256] += ok[:, 27]
    # asq row holds -0.5*(sq + BIG on pads); bsq row holds -0.5*(sq + BIG)
    sum_sq_small = sum(o[ri * CSH][0, 28] for ri in range(RSH)) * (-2.0) \
        - BIG * (R_CAP - ns)
    sum_sq_large = sum(o[ci][0, 17] for ci in range(CSH)) * (-2.0) \
        - BIG * (C_CAP - nl)
    term1_d2 = (ns * sum_sq_small - S_small @ S_small
                + nl * sum_sq_large - S_large @ S_large)
    n_same = ns * (ns - 1) / 2.0 + nl * (nl - 1) / 2.0
    return 0.5 * (term1_d2 + EPS * n_same)


def combine_term2_full(outs_list):
    """Dissimilar term from the full program's accumulators (float64)."""
    o = [np.asarray(x, np.float64) for x in outs_list]
    n_elem = float(R_CAP) * float(C_CAP)
    Ts = sum(ok[:, 0:NBLK].sum() for ok in o)          # sum of sqrt(t+eps)
    Tt = sum(ok[:, 8:8 + NBLK].sum() for ok in o)      # sum of t
    s1_hat = o[0][0, 29] / 8.0
    return 0.5 * ((Tt - n_elem) + 2.0 * (n_elem * s1_hat - Ts))


def _numpy_fallback(emb, lab):
    e = emb.astype(np.float64)
    sq = (e * e).sum(1)
    gram = e @ e.T
    d2 = np.maximum(sq[:, None] + sq[None, :] - 2.0 * gram, 0.0)
    dist = np.sqrt(d2 + EPS)
    same = (lab[:, None] == lab[None, :]).astype(np.float64)
    loss = same * 0.5 * dist ** 2 \
        + (1.0 - same) * 0.5 * np.maximum(MARGIN - dist, 0.0) ** 2
    mask = np.triu(np.ones_like(loss), k=1)
    return (loss * mask).sum() / (mask.sum() + EPS)


def run_device(in_maps, kind="detect", trace=False, **kw):
    from concourse.bass_utils import run_bass_kernel_spmd
    nc = _get_program(kind)
    names = {"detect": ("a_t", "b_t", "bsqc", "asq", "bsq"),
             "full": ("a_t", "b_t", "bsqc")}[kind]
    maps = [{k: m[k] for k in names} for m in in_maps]
    return run_bass_kernel_spmd(nc, maps, list(range(N_CORES)),
                                trace=trace, **kw)


def kernel(embeddings, labels):
    emb = np.ascontiguousarray(np.asarray(embeddings), dtype=np.float32)
    lab = np.asarray(labels).astype(np.int64).ravel()
    ok_shapes = (emb.shape == (B_FULL, D) and lab.shape == (B_FULL,)
                 and np.all((lab == 0) | (lab == 1)))
    prep = build_in_maps(emb, lab) if ok_shapes else None
    if prep is None:
        return np.float32(_numpy_fallback(emb, lab))
    in_maps, (ns, nl) = prep

    res = run_device(in_maps, kind="detect")
    outs_list = [res.results[k]["out"] for k in range(N_CORES)]
    term1 = combine_term1(outs_list, ns, nl)

    accum_max = max(float(ok[:, 0:NBLK].max()) for ok in outs_list)
    if accum_max > DETECT_ACCUM_THRESH:
        # some mixed pair is near/inside the margin: exact slow path
        res2 = run_device(in_maps, kind="full")
        term2 = combine_term2_full(
            [res2.results[k]["out"] for k in range(N_CORES)])
    else:
        term2 = 0.0

    den = B_FULL * (B_FULL - 1) / 2.0 + EPS
    return np.float32((term1 + term2) / den)
